# revision 3
# baseline (speedup 1.0000x reference)
"""BertAttention (T5-style relative-position bias) Trainium2 Bass kernel.

Strategy (8-way tensor parallel over heads, 2 heads/core):
  - Host pre-transposes hidden -> hT [HID, B*S] so QKV projection produces
    qkvT [feat, tokens] directly (feat on partitions).
  - Per core: w_qkv column slice for its 2 heads, ordered [Q_h0|Q_h1|K_h0|K_h1|V_h0|V_h1],
    Q columns pre-scaled by 1/sqrt(HD).
  - Scores computed transposed: S^T[k, q] = (K^T)^T-slice matmuls; T5 bias is added
    inside PSUM via an anti-diagonal (flip) matmul whose rhs is a Toeplitz DMA read
    of the (reversed) per-head expanded bias table; softmax without max-subtraction
    (scores are bounded, |s| < ~4); denominator via ones-column appended to V.
  - PV gives ctx^T [d, q]; normalize by reciprocal of the ones-row.
  - AllToAll reshards ctx^T from head-split to token-split; dense is computed
    transposed (out^T[e, t]) so b_dense is a per-partition bias.
  - Host reassembles out^T column chunks, transposes, reshapes.

All big matmuls run as float32r (full PE rate, ~1e-4 relative precision).
"""
import sys
import math

sys.path.insert(0, "/opt/trn_rl_repo")

import numpy as np
import ml_dtypes

import concourse.bass as bass
import concourse.bacc as bacc
import concourse.tile as tile
import concourse.mybir as mybir
from concourse.bass_utils import run_bass_kernel_spmd
from concourse.masks import make_identity

F32 = mybir.dt.float32
F32R = mybir.dt.float32r
BF16 = mybir.dt.bfloat16
Exp = mybir.ActivationFunctionType.Exp
ADD = mybir.AluOpType.add
MULT = mybir.AluOpType.mult

B, S, HID = 2, 2048, 1024
NH, HD = 16, 64
NB, MAXD = 32, 128
N_CORES = 8
HPC = NH // N_CORES          # heads per core = 2
T = B * S                    # 4096 flat tokens
FEAT = 3 * HPC * HD          # 384 qkv features per core
TC = T // 512                # 8 token chunks of 512
KTILES = S // 128            # 16 k tiles per batch
QCH = S // 512               # 4 q chunks of 512 per batch
TW = 4096                    # padded width of expanded bias table (indices 0..4094 used)


def _bucket_map_rev():
    """rev[z] = bucket(2047 - z) for z in [0, 4094], T5 bidirectional buckets."""
    rel = (2047 - np.arange(TW - 1)).astype(np.int64)   # k - q
    nb = NB // 2                                        # 16
    base = np.where(rel > 0, nb, 0)
    r = np.abs(rel)
    max_exact = nb // 2                                 # 8
    is_small = r < max_exact
    tmp = np.log(np.maximum(r, 1).astype(np.float32) / np.float32(max_exact))
    large = tmp / np.float32(math.log(MAXD / max_exact)) * np.float32(nb - max_exact)
    large_i = max_exact + large.astype(np.int32)
    large_i = np.minimum(large_i, nb - 1)
    return (base + np.where(is_small, r, large_i)).astype(np.int32)  # [4095]


def _build_program():
    nc = bacc.Bacc("TRN2", target_bir_lowering=False, debug=False,
                   enable_asserts=True, num_devices=N_CORES)

    hT_d = nc.dram_tensor("hT", [HID, T], F32R, kind="ExternalInput")
    wq_d = nc.dram_tensor("wq", [HID, FEAT], F32R, kind="ExternalInput")
    bq_d = nc.dram_tensor("bq", [FEAT, 1], F32, kind="ExternalInput")
    wd_d = nc.dram_tensor("wd", [HID, HID], F32R, kind="ExternalInput")
    bd_d = nc.dram_tensor("bd", [HID, 1], F32, kind="ExternalInput")
    tT_d = nc.dram_tensor("tT", [NB, HPC], F32R, kind="ExternalInput")
    oh_d = nc.dram_tensor("oh", [NB, TW], F32R, kind="ExternalInput")
    out_d = nc.dram_tensor("outT", [HID, T // N_CORES], F32, kind="ExternalOutput")

    with tile.TileContext(nc) as tc:
        with tc.tile_pool(name="const", bufs=1) as cst, \
             tc.tile_pool(name="big", bufs=1) as big, \
             tc.tile_pool(name="dram", bufs=1, space="DRAM") as dram:

            # ---------------- constants ----------------
            ident_f = cst.tile([128, 128], F32, tag="identf")
            make_identity(nc, ident_f[:])
            identr = cst.tile([128, 128], F32R, tag="identr")
            nc.vector.tensor_copy(identr[:], ident_f[:])
            jmat = cst.tile([128, 128], BF16, tag="jmat")
            nc.gpsimd.memset(jmat[:], 0.0)
            nc.gpsimd.affine_select(out=jmat[:], in_=jmat[:],
                                    compare_op=mybir.AluOpType.not_equal,
                                    fill=1.0, base=-127, channel_multiplier=1,
                                    pattern=[[1, 128]])
            ones_f = cst.tile([128, 1], F32, tag="ones")
            nc.gpsimd.memset(ones_f[:], 1.0)
            bq_sb = cst.tile([128, 3, 1], F32, tag="bq")
            nc.sync.dma_start(bq_sb[:], bq_d[:, :].rearrange("(m p) o -> p m o", p=128))
            bd_sb = cst.tile([128, 8, 1], F32, tag="bd")
            nc.sync.dma_start(bd_sb[:], bd_d[:, :].rearrange("(e p) o -> p e o", p=128))

            # persistent tensors, split per batch so attention on batch 0 can
            # start while batch 1's QKV projection is still running
            QTb = [big.tile([128, S], F32R, tag=f"QT{b}", name=f"QT{b}") for b in range(B)]
            KTb = [big.tile([128, S], F32R, tag=f"KT{b}", name=f"KT{b}") for b in range(B)]
            Vaugb = [big.tile([128, KTILES, 130], F32R, tag=f"Vaug{b}", name=f"Vaug{b}")
                     for b in range(B)]
            ctxTb = [big.tile([128, S], F32R, tag=f"ctxT{b}", name=f"ctxT{b}")
                     for b in range(B)]

            # ---------------- expanded bias table (device-side gather) ----------------
            trev = dram.tile([HPC, TW], BF16)
            with tc.tile_pool(name="txp", bufs=2, space="PSUM") as txp, \
                 tc.tile_pool(name="txs", bufs=1) as txs:
                tT_sb = txs.tile([NB, HPC], F32R, tag="tT")
                nc.sync.dma_start(tT_sb[:], tT_d[:, :])
                oh_sb = txs.tile([NB, TW], F32R, tag="oh")
                nc.sync.dma_start(oh_sb[:], oh_d[:, :])
                trev_sb = txs.tile([HPC, TW], BF16, tag="trevsb")
                for i in range(TW // 512):
                    tx_ps = txp.tile([HPC, 512], F32, tag="tx")
                    nc.tensor.matmul(tx_ps[:], tT_sb[:], oh_sb[:, i * 512:(i + 1) * 512],
                                     start=True, stop=True)
                    nc.vector.tensor_copy(trev_sb[:, i * 512:(i + 1) * 512], tx_ps[:])
                nc.sync.dma_start(trev[:], trev_sb[:])

            # ---------------- QKV projection ----------------
            with tc.tile_pool(name="wqp", bufs=1) as wqp, \
                 tc.tile_pool(name="htp", bufs=16) as htp, \
                 tc.tile_pool(name="vtp", bufs=1) as vtp, \
                 tc.tile_pool(name="qkvps", bufs=4, space="PSUM") as qkvps:
                wq_sb = wqp.tile([128, 8, FEAT], F32R, tag="wq")
                nc.sync.dma_start(wq_sb[:], wq_d[:, :].rearrange("(j p) f -> p j f", p=128))
                VTb = [vtp.tile([128, S], F32R, tag=f"VT{b}", name=f"VT{b}") for b in range(B)]
                with tc.tile_pool(name="trps", bufs=3, space="PSUM") as trps:
                    for b in range(B):
                        # ones columns (softmax denominator lanes) in one copy
                        nc.vector.tensor_copy(
                            Vaugb[b][:].rearrange("p t (g c) -> p t g c", c=65)[:, :, :, 64:65],
                            ones_f[:, 0:1].to_broadcast([128, KTILES, 2, 1]))
                        dests = (QTb[b], KTb[b], VTb[b])
                        for tci in range(S // 512):
                            gci = b * (S // 512) + tci
                            hts = []
                            for kt in range(8):
                                ht = htp.tile([128, 512], F32R, tag="ht")
                                nc.sync.dma_start(
                                    ht[:], hT_d[128 * kt:128 * (kt + 1),
                                                512 * gci:512 * (gci + 1)])
                                hts.append(ht)
                            for m in range(3):
                                ps = qkvps.tile([128, 512], F32, tag="qkv")
                                for kt in range(8):
                                    nc.tensor.matmul(ps[:], wq_sb[:, kt, m * 128:(m + 1) * 128],
                                                     hts[kt][:], start=(kt == 0), stop=(kt == 7))
                                nc.vector.tensor_tensor(
                                    dests[m][:, 512 * tci:512 * (tci + 1)], ps[:],
                                    bq_sb[:, m, 0:1].to_broadcast([128, 512]), ADD)

                            # ---- V transpose for this chunk's 4 token tiles ----
                            # emitted right after the chunk so each transpose only
                            # depends on the VT writes made so far (starts early,
                            # fills PE idle in the DMA-bound projection phase)
                            for t in range(4 * tci, 4 * (tci + 1)):
                                vslot = Vaugb[b][:, t, :].rearrange("p (g c) -> p g c", c=65)
                                tp = trps.tile([128, 128], F32R, tag="tr")
                                nc.tensor.transpose(tp[:], VTb[b][:, 128 * t:128 * (t + 1)],
                                                    identr[:])
                                nc.vector.tensor_copy(
                                    vslot[:, :, 0:64],
                                    tp[:].rearrange("p (g c) -> p g c", c=64))

            # ---------------- attention ----------------
            # two half-size all-to-alls, one per batch: batch 0's collective is
            # gated only on ctxTb[0] so it overlaps the tail of batch-1 attention,
            # and batch 0's dense pass overlaps batch 1's collective.
            a2a_in_b = [dram.tile([HID, S // N_CORES], F32R, name=f"a2ain{b}")
                        for b in range(B)]
            a2a_out_b = [dram.tile([HID, S // N_CORES], F32R, name=f"a2aout{b}")
                        for b in range(B)]
            wd_sb = big.tile([128, 8, HID], F32R, tag="wd")
            nc.sync.dma_start(wd_sb[:], wd_d[:, :].rearrange("(j p) e -> p j e", p=128))
            with tc.tile_pool(name="rp", bufs=40) as rp, \
                 tc.tile_pool(name="expp", bufs=4) as expp, \
                 tc.tile_pool(name="nrm", bufs=4) as nrm, \
                 tc.tile_pool(name="sps", bufs=2, space="PSUM") as sps, \
                 tc.tile_pool(name="cps", bufs=4, space="PSUM") as cps:
                for qc in range(QCH):
                    q0 = qc * 512
                    # Toeplitz bias tiles for this q chunk (shared across batches)
                    rtiles = {}
                    for kt in range(KTILES):
                        for h in range(HPC):
                            r = rp.tile([128, 512], BF16, tag="rt")
                            src = bass.AP(trev.tensor,
                                          trev.offset + h * TW + (1920 - kt * 128 + q0),
                                          [[1, 128], [1, 512]])
                            nc.sync.dma_start(r[:], src)
                            rtiles[(kt, h)] = r
                    for b in range(B):
                        ctx_ps = [cps.tile([65, 512], F32, tag="ctx", name=f"ctx{h}_{b}_{qc}")
                                  for h in range(HPC)]
                        for kt in range(KTILES):
                            k0 = kt * 128
                            s_ps = sps.tile([128, 1024], F32, tag="S")
                            # both half-width QK matmuls first (disjoint PE row
                            # groups 0-63 / 64-127 -> they run concurrently),
                            # then the full-width bias adds
                            for h in range(HPC):
                                nc.tensor.matmul(s_ps[:, 512 * h:512 * (h + 1)],
                                                 KTb[b][64 * h:64 * h + 64, k0:k0 + 128],
                                                 QTb[b][64 * h:64 * h + 64, q0:q0 + 512],
                                                 start=True, stop=False)
                            for h in range(HPC):
                                nc.tensor.matmul(s_ps[:, 512 * h:512 * (h + 1)],
                                                 jmat[:], rtiles[(kt, h)][:],
                                                 start=False, stop=True)
                            es = expp.tile([128, 1024], F32R, tag="es")
                            nc.scalar.activation(es[:], s_ps[:], Exp)
                            for h in range(HPC):
                                nc.tensor.matmul(ctx_ps[h][:],
                                                 Vaugb[b][:, kt, 65 * h:65 * h + 65],
                                                 es[:, 512 * h:512 * (h + 1)],
                                                 start=(kt == 0), stop=(kt == KTILES - 1))
                        for h in range(HPC):
                            recip = nrm.tile([1, 512], F32, tag="recip")
                            nc.vector.reciprocal(recip[:], ctx_ps[h][64:65, :])
                            rbb = nrm.tile([64, 512], F32, tag="rbb")
                            nc.gpsimd.partition_broadcast(rbb[:], recip[:])
                            nc.vector.tensor_tensor(
                                ctxTb[b][64 * h:64 * h + 64, q0:q0 + 512],
                                ctx_ps[h][0:64, :], rbb[:], MULT)
                        # stream this chunk's a2a input shards out immediately
                        nc.sync.dma_start(
                            a2a_in_b[b][:].rearrange("(j p) t -> p j t", p=128)
                            [:, 2 * qc:2 * qc + 2, :],
                            ctxTb[b][:, q0:q0 + 512].rearrange("p (j t) -> p j t", t=256))

            # ------- per-batch all-to-all (head-split -> token-split) + dense -------
            # (outside the attention pool scope: keeps peak SBUF ~19MB, well clear
            # of the ~23MB+ regime where Tile pool-reuse deps serialize phases.
            # The a2a input DMAs depend only on ctxTb so they still overlap the
            # attention tail.)
            # shard j of batch b = my ctxTb[b][:, 256-token block j]; core c
            # receives all 1024 d-dims of batch-b token block c.
            HB = S // N_CORES          # 256 tokens per core per batch
            with tc.tile_pool(name="dns", bufs=10) as dns, \
                 tc.tile_pool(name="dno", bufs=1) as dno, \
                 tc.tile_pool(name="dps", bufs=8, space="PSUM") as dps:
                outT_sb = dno.tile([128, 8, 2 * HB], F32, tag="outT")
                for b in range(B):
                    nc.gpsimd.collective_compute(
                        "AllToAll", mybir.AluOpType.bypass,
                        replica_groups=[list(range(N_CORES))],
                        ins=[a2a_in_b[b][:].opt()], outs=[a2a_out_b[b][:].opt()])
                    # j-outer / e-inner with per-slice ctx tiles: each 128KB DMA
                    # is consumed by its 8 matmuls (all 8 PSUM banks live) as
                    # soon as it lands, pipelining the post-collective DMA
                    psb = [dps.tile([128, HB], F32, tag="d", name=f"d{b}_{e}")
                           for e in range(8)]
                    for j in range(8):
                        cf = dns.tile([128, HB], F32R, tag="cf", name=f"cf{b}_{j}")
                        nc.sync.dma_start(cf[:], a2a_out_b[b][128 * j:128 * (j + 1), :])
                        for e in range(8):
                            nc.tensor.matmul(psb[e][:],
                                             wd_sb[:, j, 128 * e:128 * (e + 1)],
                                             cf[:], start=(j == 0), stop=(j == 7))
                    for e in range(8):
                        nc.vector.tensor_tensor(
                            outT_sb[:, e, b * HB:(b + 1) * HB], psb[e][:],
                            bd_sb[:, e, 0:1].to_broadcast([128, HB]), ADD)
                    nc.sync.dma_start(
                        out_d[:, :].rearrange("(e p) t -> p e t", p=128)
                        [:, :, b * HB:(b + 1) * HB],
                        outT_sb[:, :, b * HB:(b + 1) * HB])

    nc.compile()
    return nc


_NC_CACHE = None
_OH_CACHE = None


def _onehot_cached():
    """One-hot bucket map [NB, TW] — input-independent, built once per process."""
    global _OH_CACHE
    if _OH_CACHE is None:
        bm = _bucket_map_rev()
        oh = np.zeros((NB, TW), dtype=np.float32)
        oh[bm, np.arange(TW - 1)] = 1.0
        _OH_CACHE = oh
    return _OH_CACHE


def _get_program():
    global _NC_CACHE
    if _NC_CACHE is None:
        _NC_CACHE = _build_program()
    return _NC_CACHE


def _make_inmaps(hidden_states, w_qkv, b_qkv, w_dense, b_dense, rel_attn_table):
    hidden_states = np.asarray(hidden_states, dtype=np.float32)
    w_qkv = np.asarray(w_qkv, dtype=np.float32)
    b_qkv = np.asarray(b_qkv, dtype=np.float32)
    w_dense = np.asarray(w_dense, dtype=np.float32)
    b_dense = np.asarray(b_dense, dtype=np.float32)
    rel_attn_table = np.asarray(rel_attn_table, dtype=np.float32)

    hT = np.ascontiguousarray(hidden_states.reshape(T, HID).T)   # [HID, T]
    oh = _onehot_cached()

    scale = np.float32(1.0 / math.sqrt(HD))
    in_maps = []
    for c in range(N_CORES):
        ha, hb = HPC * c, HPC * c + 1
        cols = []
        bias = []
        for blk, sc in ((0, scale), (1, np.float32(1.0)), (2, np.float32(1.0))):
            for h in (ha, hb):
                sl = slice(blk * HID + h * HD, blk * HID + (h + 1) * HD)
                cols.append(w_qkv[:, sl] * sc)
                bias.append(b_qkv[sl] * sc)
        wq_c = np.ascontiguousarray(np.concatenate(cols, axis=1))        # [HID, 384]
        bq_c = np.concatenate(bias).reshape(FEAT, 1).astype(np.float32)
        in_maps.append({
            "hT": hT,
            "wq": wq_c,
            "bq": bq_c,
            "wd": w_dense,
            "bd": b_dense.reshape(HID, 1),
            "tT": np.ascontiguousarray(rel_attn_table[ha:hb + 1].T),     # [32, 2]
            "oh": oh,
        })
    return in_maps


def kernel(hidden_states, w_qkv, b_qkv, w_dense, b_dense, rel_attn_table):
    in_maps = _make_inmaps(hidden_states, w_qkv, b_qkv, w_dense, b_dense,
                           rel_attn_table)
    nc = _get_program()
    res = run_bass_kernel_spmd(nc, in_maps, core_ids=list(range(N_CORES)))
    HB = S // N_CORES
    full = np.empty((HID, T), dtype=np.float32)
    for c in range(N_CORES):
        o = res.results[c]["outT"]            # [HID, 2*HB]: [b0 block c | b1 block c]
        full[:, c * HB:(c + 1) * HB] = o[:, :HB]
        full[:, S + c * HB:S + (c + 1) * HB] = o[:, HB:]
    return np.ascontiguousarray(full.T).reshape(B, S, HID)

